# revision 41
# baseline (speedup 1.0000x reference)
"""Trainium2 Bass kernel for the AMTCL loss (nn_AMTCL_66520453480770).

Math: the reference's [B,B] pairwise-distance mining collapses to the [B,C]
matrix dc2[i,c] = sum_d w2[c,d]*(centers[c,d]-inputs[i,d])**2 because
dist[i,j] depends on j only through c = targets[j]:
    ap2[i] = dc2[i, t_i]
    an2[i] = min_{c present, c != t_i} dc2[i,c]
    cc2[i] = cdmin2[t_i],  cdmin2[c] = max(min_{j != c} cd2[c,j], 0)
    loss_i = sqrt(ap2) + relu(cc - an)
           = sqrt(ap2) + sqrt(cc2) - sqrt(min(an2, cc2))   (sqrt monotone)

Device GEMM chain per 128-anchor chunk (PSUM f32):
    dc2 = xsq @ w2.T + x @ m2.T + [PEN_OH*onehot + arow]
where the bracket is ONE matmul: lhsT = [ohT; ones-row] (fp8, 101 x 128),
rhs = epa = [PEN_OH*I; arow] (bf16, 101 x 100).  Mining is then just two DVE
reduces straight out of PSUM: an2 = min_c, ap2 = max_c - PEN_OH (PEN_OH=2^22
keeps f32 ulp at 0.5; the -PEN_OH rides the final Sqrt activation's bias).
Absent classes carry +PEN_ABS=2^20 inside arow (> any dc2, < PEN_OH so the
max still finds the self column). cd2 [C,C] reuses the tables plus one
[eyepen;arow] x [eye;ones] matmul; cc2 is gathered per anchor chunk by a
tiny matmul (lhsT=ohT fp8, rhs=cdmin2 as bf16 column).

All large inputs ride in ONE fp8 (e4m3) tensor — w2T | m2T | ohT | x01 |
cT | x23 — because the DMA engines cost ~100ns per partition-row
descriptor regardless of size: one merged tensor in two dma_starts is 258
descriptors instead of 500+ for separate bf16 tensors. fp8 quantization of
the tables perturbs the 769-term distance sums by only ~0.3% (validated
1.8e-4 end-to-end loss error). Squares of fp8 are exact in bf16; mixed
fp8 x bf16 matmuls are exact. The bf16 arow row (exact f32 a-sums) lands
via a 2-descriptor DMA straight into partition 100 of the epa tile.

The output is matmul-reduced to [1,12] column sums so the out-DMA is a
single descriptor. The scalar engine needs only the sqrt table set
(square/relu/sqrt share one), loaded once via a dummy sqrt. Wide warmup
matmuls keep the PE busy until data lands — the PE clocks up from 1.2 to
2.4 GHz only after ~3us of near-continuous work, which halves matmul time
for the later chunks.

Host work is O(C*D) table prep / index packing plus the final unshard:
sum cols 0:8 minus cols 8:12 of the per-core [1,12] outputs, divide by B.
"""

import ml_dtypes
import numpy as np

import concourse.bass as bass
import concourse.bacc as bacc
import concourse.mybir as mybir
import concourse.tile as tile
from concourse.bass_utils import run_bass_kernel_spmd

B, C, D = 4096, 100, 384
NCORES = 8
ROWS = B // NCORES          # 512 anchor rows per core
MCH = ROWS // 128           # 4 partition chunks of anchor rows
KD = D // 128               # 3 partition chunks of the feature dim
PEN_OH = float(2 ** 22)     # one-hot / diagonal penalty (rides sqrt bias)
PEN_ABS = float(2 ** 20)    # absent-class penalty (baked into arow)
F32 = mybir.dt.float32
BF16 = mybir.dt.bfloat16
FP8 = mybir.dt.float8e4
AF = mybir.ActivationFunctionType
ALU = mybir.AluOpType

NWARM = 5                   # [128,512] warmup matmuls before data lands

# xoh column layout (fp8), ordered by need time; split point after x1.
# Chunks 0/1 are squared on device (DVE/scalar, idle early); chunks 2/3
# bring host-squared fp8 xsq in the second dma so no square gates them.
W2_O = 0                    # w2T, KD chunks of C cols
M2_O = KD * C               # m2T
OH_O = 2 * KD * C           # ohT (+ones row 100), MCH chunks of 128
X01_O = OH_O + MCH * 128    # x chunks 0..1
CT_O = X01_O + 2 * D        # cT
X23_O = CT_O + KD * C       # x2|xsq2|x3|xsq3
XOHW = X23_O + 4 * D        # 3716
XSPLIT = CT_O               # first dma: w2T|m2T|ohT|x0|x1


def _xoff(m):
    return (X01_O + m * D) if m < 2 else (X23_O + 2 * (m - 2) * D)


def _qoff(m):
    return _xoff(m) + D     # host xsq slot, chunks 2..3 only


def build_nc() -> bass.Bass:
    nc = bacc.Bacc(
        "TRN2", target_bir_lowering=False, debug=False, num_devices=NCORES
    )

    xoh_d = nc.declare_dram_parameter("xoh", [128, XOHW], FP8, isOutput=False)
    arow_d = nc.declare_dram_parameter("arow", [2, C], BF16, isOutput=False)
    out_d = nc.declare_dram_parameter("out", [128, 12], F32, isOutput=True)

    with tile.TileContext(nc) as tc:
        with (
            tc.tile_pool(name="wts", bufs=1) as wp,
            tc.tile_pool(name="ps1", bufs=1, space="PSUM") as pp1,
            tc.tile_pool(name="ps2", bufs=1, space="PSUM") as pp2,
        ):
            # ---- DMAs: descgen split across the two HWDGE queues ----
            xoh = wp.tile([128, XOHW], FP8, tag="xoh")
            nc.sync.dma_start(xoh[:, 0:XSPLIT], xoh_d[:, 0:XSPLIT])
            epa = wp.tile([101, C], BF16, tag="epa")
            nc.sync.dma_start(epa[100:101, :], arow_d[0:1, :])
            eyeone = wp.tile([101, C], BF16, tag="eyeone")
            nc.sync.dma_start(eyeone[100:101, :], arow_d[1:2, :])
            nc.scalar.dma_start(xoh[:, XSPLIT:], xoh_d[:, XSPLIT:])

            w2t = xoh[:, W2_O : W2_O + KD * C]
            m2t = xoh[:, M2_O : M2_O + KD * C]
            ctt = xoh[:, CT_O : CT_O + KD * C]

            # ---- constants (no input deps); warm_b on DVE: starts sooner
            warm_b = wp.tile([128, 512], BF16, tag="warm_b")
            nc.vector.memset(warm_b[:], 1.0)
            dums = wp.tile([1, 1], F32, tag="dums")
            nc.gpsimd.memset(dums[:], 1.0)
            negpen = wp.tile([128, 1], F32, tag="negpen")
            nc.gpsimd.memset(negpen[:], -PEN_OH)
            penb = wp.tile([C, C], BF16, tag="penb")
            nc.gpsimd.memset(penb[:], PEN_OH)
            nc.gpsimd.affine_select(
                eyeone[0:C, :], warm_b[0:C, 0:C], pattern=[[1, C]],
                compare_op=ALU.is_equal, fill=0.0, base=0,
                channel_multiplier=-1,
            )
            nc.gpsimd.affine_select(
                epa[0:C, :], penb[:], pattern=[[1, C]],
                compare_op=ALU.is_equal, fill=0.0, base=0,
                channel_multiplier=-1,
            )
            # center squares for the cd2 quad term (exact: fp8^2 in bf16)
            csqt = wp.tile([128, KD * C], BF16, tag="csqt")
            nc.gpsimd.tensor_tensor(csqt[:], ctt, ctt, op=ALU.mult)

            # ---- scalar: sqrt-table preload (square/relu/sqrt one set) ----
            dumsq = wp.tile([1, 1], F32, tag="dumsq")
            nc.scalar.sqrt(dumsq[:], dums[:])

            # ---- PE: p-state warmup until real operands land ----
            warm_ps = pp1.tile([128, 512], F32, tag="warm")
            for i in range(NWARM):
                nc.tensor.matmul(
                    warm_ps[:], warm_b[:, 0:128], warm_b[:],
                    start=(i == 0), stop=(i == NWARM - 1),
                )

            xsq = wp.tile([128, 2 * D], BF16, tag="xsq")

            def sq(eng, m):
                xl = slice(_xoff(m), _xoff(m) + D)
                sl = slice(m * D, (m + 1) * D)
                if eng is nc.vector:
                    eng.tensor_tensor(xsq[:, sl], xoh[:, xl], xoh[:, xl],
                                      op=ALU.mult)
                else:
                    eng.square(xsq[:, sl], xoh[:, xl])

            psum_dc2 = []
            for m in range(MCH):
                psum_dc2.append(
                    pp2.tile([128, C], F32, name=f"dc2_{m}", tag=f"dc2_{m}")
                )

            def chunk_x_mms(m):
                for k in range(KD):
                    nc.tensor.matmul(
                        psum_dc2[m][:],
                        xoh[:, _xoff(m) + k * 128 : _xoff(m) + (k + 1) * 128],
                        m2t[:, k * C : (k + 1) * C],
                        start=(k == 0), stop=False,
                    )

            def chunk_xsq_mms(m):
                for k in range(KD):
                    if m < 2:
                        xq = xsq[:, m * D + k * 128 : m * D + (k + 1) * 128]
                    else:
                        xq = xoh[:, _qoff(m) + k * 128 :
                                    _qoff(m) + (k + 1) * 128]
                    nc.tensor.matmul(
                        psum_dc2[m][:], xq,
                        w2t[:, k * C : (k + 1) * C],
                        start=False, stop=False,
                    )

            def chunk_pen_mm(m):
                nc.tensor.matmul(
                    psum_dc2[m][:],
                    xoh[0:101, OH_O + m * 128 : OH_O + (m + 1) * 128],
                    epa[:], start=False, stop=True,
                )

            an2all = wp.tile([128, MCH], F32, tag="an2all")
            tail = wp.tile([128, 12], F32, tag="tail")

            def mine(m):
                nc.vector.tensor_reduce(
                    an2all[:, m : m + 1], psum_dc2[m][:],
                    axis=mybir.AxisListType.X, op=ALU.min,
                )
                nc.vector.tensor_reduce(
                    tail[:, m : m + 1], psum_dc2[m][:],
                    axis=mybir.AxisListType.X, op=ALU.max,
                )

            # ---- squares: chunk 0 on DVE (mining engine stays free
            # afterwards), chunk 1 on the otherwise-idle scalar ----
            sq(nc.vector, 0)
            sq(nc.scalar, 1)

            # ---- PE stream (ready-order to avoid FIFO stalls) ----
            psum_cd2 = pp1.tile([C, C], F32, tag="cd2")
            chunk_x_mms(0)
            chunk_xsq_mms(0)
            chunk_pen_mm(0)
            chunk_x_mms(1)
            chunk_xsq_mms(1)
            chunk_pen_mm(1)
            for k in range(KD):          # cd2 cross terms (need cT: 2nd dma)
                nc.tensor.matmul(
                    psum_cd2[:], m2t[:, k * C : (k + 1) * C],
                    ctt[:, k * C : (k + 1) * C],
                    start=(k == 0), stop=False,
                )
            for k in range(KD):          # cd2 quad terms (need csqt)
                nc.tensor.matmul(
                    psum_cd2[:], w2t[:, k * C : (k + 1) * C],
                    csqt[:, k * C : (k + 1) * C],
                    start=False, stop=False,
                )
            nc.tensor.matmul(            # cd2 diag penalty + arow rank-1
                psum_cd2[:], epa[:], eyeone[:], start=False, stop=True,
            )
            chunk_x_mms(2)
            chunk_xsq_mms(2)
            chunk_pen_mm(2)
            chunk_x_mms(3)
            chunk_xsq_mms(3)
            chunk_pen_mm(3)

            # ---- DVE: mining + cd2 min chain ----
            mine(0)
            mine(1)
            cdmin2 = wp.tile([C, 1], F32, tag="cdmin2")
            nc.vector.tensor_reduce(
                cdmin2[:], psum_cd2[:], axis=mybir.AxisListType.X, op=ALU.min
            )
            cdminb = wp.tile([C, 1], BF16, tag="cdminb")
            nc.vector.tensor_scalar(cdminb[:], cdmin2[:], 0.0, None,
                                    op0=ALU.max)
            mine(2)
            mine(3)

            # ---- PE: cc2 gathers (lhsT=ohT fp8, rhs=cdmin2 bf16 col) ----
            psum_cc2 = pp1.tile([128, MCH], F32, tag="cc2")
            for m in range(MCH):
                nc.tensor.matmul(
                    psum_cc2[:, m : m + 1],
                    xoh[0:C, OH_O + m * 128 : OH_O + (m + 1) * 128],
                    cdminb[:], start=True, stop=True,
                )

            # ---- tail: loss_i = sqrt(ap2) + sqrt(cc2) - sqrt(min) ----
            # ap sqrts run per chunk as each max lands (scalar is idle);
            # the final gate is just ttmin -> one [128,8] sqrt -> out dma.
            nc.vector.tensor_copy(tail[:, 4:8], psum_cc2[:])
            nc.vector.tensor_tensor(
                tail[:, 8:12], an2all[:], psum_cc2[:], op=ALU.min
            )
            tailsq = wp.tile([128, 12], F32, tag="tailsq")
            for m in range(MCH):
                nc.scalar.activation(tailsq[:, m : m + 1], tail[:, m : m + 1],
                                     AF.Sqrt, bias=negpen[:])
            nc.scalar.activation(tailsq[:, 4:12], tail[:, 4:12], AF.Sqrt)
            nc.scalar.dma_start(out_d[:], tailsq[:])

    nc.compile()
    return nc


_NC_CACHE: list = []


def _get_nc() -> bass.Bass:
    if not _NC_CACHE:
        _NC_CACHE.append(build_nc())
    return _NC_CACHE[0]


def make_in_maps(inputs, centers, centers_weights, targets):
    x = np.asarray(inputs, dtype=np.float32)
    c = np.asarray(centers, dtype=np.float32)
    cw = np.asarray(centers_weights, dtype=np.float32)
    t = np.asarray(targets).astype(np.int64)
    bf = ml_dtypes.bfloat16
    f8 = ml_dtypes.float8_e4m3

    w2 = 2.0 ** cw                                      # [C, D] f32
    m2 = -2.0 * w2 * c                                  # [C, D] f32

    base = np.zeros((128, XOHW), dtype=np.float32)
    for k in range(KD):
        sl = slice(k * 128, (k + 1) * 128)
        base[:, W2_O + k * C : W2_O + (k + 1) * C] = w2.T[sl]
        base[:, M2_O + k * C : M2_O + (k + 1) * C] = m2.T[sl]
        base[:, CT_O + k * C : CT_O + (k + 1) * C] = c.T[sl]

    present = np.zeros(C, dtype=bool)
    present[np.unique(t)] = True
    arow = np.ones((2, C), dtype=np.float32)            # row 1: ones row
    arow[0] = (w2 * c * c).sum(axis=1) + PEN_ABS * (~present)
    arow = arow.astype(bf)

    # quantize x once so host xsq == (device fp8 x)^2
    xT = np.ascontiguousarray(x.T).astype(f8).astype(np.float32)  # [D, B]

    in_maps = []
    for i in range(NCORES):
        rows = slice(i * ROWS, (i + 1) * ROWS)
        xoh = base.copy()
        # [m, p, k*128+j]: anchor-chunk-major packing of x.T
        xr = xT[:, rows].reshape(KD, 128, MCH, 128).transpose(2, 1, 0, 3)
        xr = xr.reshape(MCH, 128, KD * 128)
        for m in range(MCH):
            xoh[:, _xoff(m) : _xoff(m) + D] = xr[m]
            if m >= 2:
                xoh[:, _qoff(m) : _qoff(m) + D] = xr[m] * xr[m]
        ts = t[rows].reshape(MCH, 128)
        for m in range(MCH):
            xoh[:C, OH_O + m * 128 : OH_O + (m + 1) * 128] = (
                np.arange(C)[:, None] == ts[m][None, :]
            )
        xoh[C:, OH_O : OH_O + MCH * 128] = 0.0
        xoh[100, OH_O : OH_O + MCH * 128] = 1.0         # arow ones row
        in_maps.append({
            "xoh": xoh.astype(f8),
            "arow": arow,
        })
    return in_maps


def kernel(inputs, centers, centers_weights, targets, epoch_number=None,
           **_ignored):
    nc = _get_nc()
    in_maps = make_in_maps(inputs, centers, centers_weights, targets)
    res = run_bass_kernel_spmd(nc, in_maps, core_ids=list(range(NCORES)))
    total = 0.0
    for r in res.results:
        o = np.asarray(r["out"], dtype=np.float64)
        total += o[:, 0:8].sum() - o[:, 8:12].sum()
    return np.float32(total / B)
